# revision 4
# baseline (speedup 1.0000x reference)
"""ConfidenceBiasedCrossAttention Trainium2 kernel (8 NeuronCores).

Sharding (Megatron-style, per spec hint): data-parallel over B (2) x
head-parallel over 4 head-groups of 4 heads (256 channels) -> 8 cores.
Each core computes q/k/v projections for its 256 channels, biased
softmax attention for its 4 heads, and a partial output projection
(rows of Wo). Host sums the 4 partials per batch and adds Wo_b.

Device pipeline per core (all matmuls in fp32r = fast fp32):
  - PE-transpose Q/K/V blocks and weight slices (fp32, exact)
  - qT/kT [256ch, L] and v [Lk, 256ch(+ones col)] projections
  - logitsT [Lk-chunk, Lq] = k_h qT_h; exp(0.125*x + V_bias[k]) on ACT
  - [attn_out.T | denom] accumulated over Lk chunks in PSUM via the
    ones column; normalize with reciprocal + K=1 broadcast matmul
  - partial out = attnT.T @ WoT accumulated over the 256 channels
"""

import numpy as np

import concourse.bacc as bacc
import concourse.mybir as mybir
import concourse.tile as tile
from concourse import bass_utils
from concourse.masks import make_identity

F32 = mybir.dt.float32
F32R = mybir.dt.float32r
AF = mybir.ActivationFunctionType
MUL = mybir.AluOpType.mult
ADD = mybir.AluOpType.add

P = 128
C = 1024
H = 16
D = 64
LQ = 1024
LK = 4096
CS = 256          # channels per core (4 heads)
NH = 4            # heads per core
SCALE = 1.0 / 8.0
BLK = 256         # Lk/Lq rows per processing block
NKB = LK // BLK   # 16
NQB = LQ // BLK   # 4
NCH = LK // P     # 32 Lk chunks of 128


def _transpose_block(nc, ps, ident, dst, src_block, n_sub):
    """Transpose n_sub [128,128] blocks of src (natural [row, ch]) into dst
    [128(ch-chunk), n_sub*128(rows)] (f32r) via PSUM."""
    pt = ps.tile([P, n_sub * P], F32, tag="t")
    for a in range(n_sub):
        nc.tensor.transpose(pt[:, a * P : (a + 1) * P], src_block[a], ident)
    nc.vector.tensor_copy(dst, pt)


def build_nc():
    nc = bacc.Bacc("TRN2", target_bir_lowering=False, debug=False, num_devices=8)
    qb_d = nc.dram_tensor("Qb", [LQ, C], F32, kind="ExternalInput").ap()
    kb_d = nc.dram_tensor("Kb", [LK, C], F32, kind="ExternalInput").ap()
    vb_d = nc.dram_tensor("Vb", [LK, C], F32, kind="ExternalInput").ap()
    vbias_d = nc.dram_tensor("vbias", [P, NCH], F32, kind="ExternalInput").ap()
    wq_d = nc.dram_tensor("wq", [CS, C], F32, kind="ExternalInput").ap()
    wk_d = nc.dram_tensor("wk", [CS, C], F32, kind="ExternalInput").ap()
    wv_d = nc.dram_tensor("wv", [CS, C], F32, kind="ExternalInput").ap()
    wo_d = nc.dram_tensor("wo", [C, CS], F32, kind="ExternalInput").ap()
    bq_d = nc.dram_tensor("bq", [P, 2], F32, kind="ExternalInput").ap()
    bk_d = nc.dram_tensor("bk", [P, 2], F32, kind="ExternalInput").ap()
    bv_d = nc.dram_tensor("bv", [1, CS], F32, kind="ExternalInput").ap()
    out_d = nc.dram_tensor("out", [LQ, C], F32, kind="ExternalOutput").ap()

    with tile.TileContext(nc) as tc:
        with (
            tc.tile_pool(name="pers", bufs=1) as pers,
            tc.tile_pool(name="sb", bufs=1) as sb,
            tc.tile_pool(name="ps", bufs=2, space="PSUM") as ps,
        ):
            # ---- constants ----
            ident = pers.tile([P, P], F32)
            make_identity(nc, ident)
            ones_f32 = pers.tile([P, 1], F32)
            nc.gpsimd.memset(ones_f32, 1.0)
            ones_r = pers.tile([1, P], F32R)
            nc.vector.tensor_copy(ones_r, ones_f32[0:1, :].to_broadcast([1, P]))
            vbias_sb = pers.tile([P, NCH], F32)
            nc.sync.dma_start(vbias_sb, vbias_d)
            bq_sb = pers.tile([P, 2], F32)
            nc.sync.dma_start(bq_sb, bq_d)
            bk_sb = pers.tile([P, 2], F32)
            nc.sync.dma_start(bk_sb, bk_d)
            bv_st = pers.tile([1, CS], F32)
            nc.sync.dma_start(bv_st, bv_d)
            # bv must be "rounded f32r" before feeding the bcast matmul
            bv_row = pers.tile([1, CS], F32R)
            nc.vector.tensor_copy(bv_row, bv_st)
            # broadcast v-bias to all 128 partitions via K=1 matmul
            pvb = ps.tile([P, CS], F32, tag="t")
            nc.tensor.matmul(pvb, ones_r[:, 0:P], bv_row, start=True, stop=True)
            vb_bc = pers.tile([P, CS], F32)
            nc.vector.tensor_copy(vb_bc, pvb)

            # ---- persistent activations ----
            qT = pers.tile([P, 2, LQ], F32R)       # [ch%128, ch//128, Lq]
            kT = pers.tile([P, 2, LK], F32R)
            v65 = pers.tile([P, NCH, NH, D + 1], F32R)  # [k%128, chunk, h, v|1]
            attnT = pers.tile([P, 2, LQ], F32R)
            wqT = pers.tile([P, 8, CS], F32R)      # [cin%128, cin//128, cout]
            wkT = pers.tile([P, 8, CS], F32R)
            wvT = pers.tile([P, 8, CS], F32R)
            woT = pers.tile([P, 2, C], F32R)       # [ch%128, ch//128, cout]

            # ones column of v65 (denominator trick)
            nc.vector.tensor_copy(
                v65[:, :, :, D].rearrange("p a b -> p (a b)"),
                ones_f32.to_broadcast([P, NCH * NH]),
            )

            # ---- weight transposes ----
            for w_d_, wT in ((wq_d, wqT), (wk_d, wkT), (wv_d, wvT)):
                w_nat = sb.tile([P, 2, C], F32, tag="wnat", bufs=2)
                nc.sync.dma_start(w_nat, w_d_.rearrange("(t p) c -> p t c", p=P))
                for i in range(8):
                    _transpose_block(
                        nc, ps, ident, wT[:, i, :],
                        [w_nat[:, mt, i * P : (i + 1) * P] for mt in range(2)], 2,
                    )
            wo_nat = sb.tile([P, 8, CS], F32, tag="wnat", bufs=2)
            nc.sync.dma_start(wo_nat, wo_d.rearrange("(t p) c -> p t c", p=P))
            for kc in range(2):
                pw = ps.tile([P, C], F32, tag="big")
                for j in range(8):
                    nc.tensor.transpose(
                        pw[:, j * P : (j + 1) * P],
                        wo_nat[:, j, kc * P : (kc + 1) * P],
                        ident,
                    )
                nc.vector.tensor_copy(woT[:, kc, :], pw)

            # ---- Q projection (4 blocks of 256 rows) ----
            for blk in range(NQB):
                qin = sb.tile([P, 2, C], F32, tag="xin", bufs=2)
                nc.sync.dma_start(
                    qin,
                    qb_d[blk * BLK : (blk + 1) * BLK, :].rearrange(
                        "(t p) c -> p t c", p=P
                    ),
                )
                xt = sb.tile([P, 8, BLK], F32R, tag="xt", bufs=2)
                for i in range(8):
                    _transpose_block(
                        nc, ps, ident, xt[:, i, :],
                        [qin[:, a, i * P : (i + 1) * P] for a in range(2)], 2,
                    )
                for mt in range(2):
                    pq = ps.tile([P, BLK], F32, tag="big")
                    for i in range(8):
                        nc.tensor.matmul(
                            pq, wqT[:, i, mt * P : (mt + 1) * P], xt[:, i, :],
                            start=(i == 0), stop=(i == 7),
                        )
                    nc.scalar.activation(
                        qT[:, mt, blk * BLK : (blk + 1) * BLK], pq,
                        AF.Identity, bias=bq_sb[:, mt : mt + 1],
                    )

            # ---- K & V projections, interleaved blocks of 256 rows ----
            for blk in range(NKB):
                kin = sb.tile([P, 2, C], F32, tag="xin", bufs=2)
                nc.sync.dma_start(
                    kin,
                    kb_d[blk * BLK : (blk + 1) * BLK, :].rearrange(
                        "(t p) c -> p t c", p=P
                    ),
                )
                xt = sb.tile([P, 8, BLK], F32R, tag="xt", bufs=2)
                for i in range(8):
                    _transpose_block(
                        nc, ps, ident, xt[:, i, :],
                        [kin[:, a, i * P : (i + 1) * P] for a in range(2)], 2,
                    )
                for mt in range(2):
                    pk = ps.tile([P, BLK], F32, tag="big")
                    for i in range(8):
                        nc.tensor.matmul(
                            pk, wkT[:, i, mt * P : (mt + 1) * P], xt[:, i, :],
                            start=(i == 0), stop=(i == 7),
                        )
                    nc.scalar.activation(
                        kT[:, mt, blk * BLK : (blk + 1) * BLK], pk,
                        AF.Identity, bias=bk_sb[:, mt : mt + 1],
                    )

                vin = sb.tile([P, 2, C], F32, tag="xin", bufs=2)
                nc.sync.dma_start(
                    vin,
                    vb_d[blk * BLK : (blk + 1) * BLK, :].rearrange(
                        "(t p) c -> p t c", p=P
                    ),
                )
                xtv = sb.tile([P, 8, BLK], F32R, tag="xt", bufs=2)
                for i in range(8):
                    _transpose_block(
                        nc, ps, ident, xtv[:, i, :],
                        [vin[:, a, i * P : (i + 1) * P] for a in range(2)], 2,
                    )
                for a in range(2):
                    pv = ps.tile([P, CS], F32, tag="big")
                    for i in range(8):
                        nc.tensor.matmul(
                            pv, xtv[:, i, a * P : (a + 1) * P], wvT[:, i, :],
                            start=(i == 0), stop=(i == 7),
                        )
                    nc.vector.tensor_tensor(
                        v65[:, blk * 2 + a, :, 0:D],
                        pv.rearrange("p (h d) -> p h d", d=D),
                        vb_bc.rearrange("p (h d) -> p h d", d=D),
                        ADD,
                    )

            # ---- attention, per head ----
            for h in range(NH):
                ht, hp = h // 2, (h % 2) * D
                po = ps.tile([P, LQ], F32, tag="acc", bufs=1)
                for c in range(NCH):
                    pl = ps.tile([P, LQ], F32, tag="big")
                    for n in range(2):
                        nc.tensor.matmul(
                            pl[:, n * 512 : (n + 1) * 512],
                            kT[hp : hp + D, ht, c * P : (c + 1) * P],
                            qT[hp : hp + D, ht, n * 512 : (n + 1) * 512],
                            start=True, stop=True, tile_position=(hp, 0),
                        )
                    eT = sb.tile([P, LQ], F32R, tag="exp", bufs=3)
                    nc.scalar.activation(
                        eT, pl, AF.Exp, bias=vbias_sb[:, c : c + 1], scale=SCALE
                    )
                    for n in range(2):
                        nc.tensor.matmul(
                            po[0 : D + 1, n * 512 : (n + 1) * 512],
                            v65[:, c, h, :],
                            eT[:, n * 512 : (n + 1) * 512],
                            start=(c == 0), stop=(c == NCH - 1),
                        )
                rec = sb.tile([1, LQ], F32R, tag="rec")
                with nc.allow_low_precision(reason="softmax denom reciprocal"):
                    nc.vector.reciprocal(rec, po[D : D + 1, :])
                pb = ps.tile([P, LQ], F32, tag="big")
                for n in range(2):
                    nc.tensor.matmul(
                        pb[0:D, n * 512 : (n + 1) * 512],
                        ones_r[:, 0:D],
                        rec[:, n * 512 : (n + 1) * 512],
                        start=True, stop=True,
                    )
                bc = sb.tile([D, LQ], F32, tag="bc")
                nc.vector.tensor_copy(bc, pb[0:D, :])
                nc.vector.tensor_tensor(
                    attnT[hp : hp + D, ht, :], po[0:D, :], bc, MUL
                )

            # ---- output projection (partial; host adds bias + reduces) ----
            for m in range(8):
                pw = ps.tile([P, C], F32, tag="big")
                for kc in range(2):
                    for n in range(2):
                        nc.tensor.matmul(
                            pw[:, n * 512 : (n + 1) * 512],
                            attnT[:, kc, m * P : (m + 1) * P],
                            woT[:, kc, n * 512 : (n + 1) * 512],
                            start=(kc == 0), stop=(kc == 1),
                        )
                ob = sb.tile([P, C], F32, tag="ob", bufs=2)
                nc.vector.tensor_copy(ob, pw)
                nc.sync.dma_start(out_d[m * P : (m + 1) * P, :], ob)

    nc.compile()
    return nc


_NC = None


def _get_nc():
    global _NC
    if _NC is None:
        _NC = build_nc()
    return _NC


def shard_inputs(Q, K_in, V_in, V_bias, Wq_w, Wq_b, Wk_w, Wk_b, Wv_w, Wv_b, Wo_w, Wo_b):
    """Build the 8 per-core input dicts."""
    in_maps = []
    for core in range(8):
        b, g = core // 4, core % 4
        gs, ge = g * CS, (g + 1) * CS
        in_maps.append({
            "Qb": np.ascontiguousarray(Q[b]),
            "Kb": np.ascontiguousarray(K_in[b]),
            "Vb": np.ascontiguousarray(V_in[b]),
            "vbias": np.ascontiguousarray(V_bias[b].reshape(NCH, P).T),
            "wq": np.ascontiguousarray(Wq_w[gs:ge]),
            "wk": np.ascontiguousarray(Wk_w[gs:ge]),
            "wv": np.ascontiguousarray(Wv_w[gs:ge]),
            "wo": np.ascontiguousarray(Wo_w[:, gs:ge]),
            "bq": np.ascontiguousarray(Wq_b[gs:ge].reshape(2, P).T),
            "bk": np.ascontiguousarray(Wk_b[gs:ge].reshape(2, P).T),
            "bv": np.ascontiguousarray(Wv_b[gs:ge].reshape(1, CS)),
        })
    return in_maps


def combine_outputs(results, Wo_b):
    """Sum the 4 head-group partials per batch and add the output bias."""
    outs = np.stack([r["out"] for r in results]).reshape(2, 4, LQ, C)
    return (outs.sum(axis=1) + Wo_b[None, None, :]).astype(np.float32)


def kernel(**inputs):
    nc = _get_nc()
    in_maps = shard_inputs(**inputs)
    res = bass_utils.run_bass_kernel_spmd(nc, in_maps, core_ids=list(range(8)))
    return combine_outputs(res.results, np.asarray(inputs["Wo_b"]))


if __name__ == "__main__":
    rng = np.random.default_rng(0)
    ins = {
        "Q": rng.standard_normal((2, LQ, C), dtype=np.float32),
        "K_in": rng.standard_normal((2, LK, C), dtype=np.float32),
        "V_in": rng.standard_normal((2, LK, C), dtype=np.float32),
        "V_bias": rng.standard_normal((2, LK)).astype(np.float32),
        **{
            f"W{x}_w": (rng.standard_normal((C, C)) * 0.03).astype(np.float32)
            for x in "qkvo"
        },
        **{
            f"W{x}_b": (rng.standard_normal(C) * 0.03).astype(np.float32)
            for x in "qkvo"
        },
    }
    out = kernel(**ins)
    print("ok", out.shape, out.dtype)
